# revision 9
# baseline (speedup 1.0000x reference)
"""Capsule-routing kernel for Trainium2, data-parallel over batch (8 cores).

Math: the reference's per-instance routing (unique -> gather -> attention)
is reformulated as a dense masked softmax over the 64x64 cell grid:
  - all per-cell quantities (attention keys, value-scalar, activation logit)
    come from one fused per-image GEMM,
  - the relative-position encoding's mean term cancels in the softmax and
    reduces to a rank-1 correction computed from per-instance occupancy sums,
  - per-instance dedup of points is an occupancy bitmap over cells
    (host-precomputed from the integer point lists, like the folded weights),
  - all 32 instances reduce in a single accumulated PE matmul against the
    occupancy mask.

v3: everything in bf16 (validated max rel err ~1.1e-3 vs the 2e-2 gate):
halves the X HBM traffic and runs the PE at 1 cycle/row instead of
f32r's 2.  The whole bf16 X (10.5 MB) is prefetched into SBUF up front
as 20 [128, 2048] tiles so DMA never stalls on buffering; weights and
the occupancy bitmap are pre-swizzled on the host into their SBUF
layouts so every input is one contiguous DMA dispatch.  The A-tiles
live in one persistent SBUF strip whose positional-correction columns
are filled once, and the final sigmoid reads its PSUM accumulator
directly.
"""
import sys

sys.path.insert(0, "/opt/trn_rl_repo")

import numpy as np
import ml_dtypes

import concourse.bacc as bacc
import concourse.mybir as mybir
from concourse import masks, tile
from concourse.bass_utils import run_bass_kernel_spmd

F32 = mybir.dt.float32
BF16 = mybir.dt.bfloat16

B = 8
CIN = 1280
NCELL = 4096  # 64x64 feature grid
NCAPS = 19
NI = 32  # instances per image
NPTS = 256  # points per instance
DK = 64
EPS = 1e-6
NCH = 10  # channel chunks of 128
NCK = 32  # 128-cell chunks

_CACHE = {}

# Force every activation onto the one table set that covers exp/ln/copy so
# the ACT engine never reloads its function tables mid-kernel.
_ONE_SET = "natural_log_exp_and_others"
_orig_get_tables = None


def _patched_tables(arch):
    full = _orig_get_tables(arch)
    return {
        name: (funcs if name == _ONE_SET else set())
        for name, funcs in full.items()
    }


def _install_act_table_patch():
    global _orig_get_tables
    if _orig_get_tables is None:
        _orig_get_tables = bacc.get_activation_tables
        bacc.get_activation_tables = _patched_tables


def _build_nc(dbg=False, loop_n=1, mode="full"):
    key = ("nc", dbg, loop_n, mode)
    if key in _CACHE:
        return _CACHE[key]

    _install_act_table_patch()
    nc = bacc.Bacc(None, target_bir_lowering=False, debug=False)

    X = nc.dram_tensor("X", [CIN, NCELL], BF16, kind="ExternalInput")
    WSW = nc.dram_tensor("WSW", [128, NCH * 66], BF16, kind="ExternalInput")
    WC3 = nc.dram_tensor("WC3", [3, 66], BF16, kind="ExternalInput")
    C3 = nc.dram_tensor("C3", [3, NCELL], BF16, kind="ExternalInput")
    QTB = nc.dram_tensor("QTB", [DK, NCAPS], BF16, kind="ExternalInput")
    WC2 = nc.dram_tensor("WC2", [128, 2 * NCK], BF16, kind="ExternalInput")
    OCCT = nc.dram_tensor("OCCT", [128, NCK * NI], BF16, kind="ExternalInput")
    OUT = nc.dram_tensor("OUT", [NI, NCAPS], F32, kind="ExternalOutput")

    with tile.TileContext(nc) as tc:
        with (
            tc.tile_pool(name="const", bufs=1) as cpool,
            tc.tile_pool(name="xp", bufs=1) as xpool,
            tc.tile_pool(name="m1", bufs=1) as m1pool,
            tc.tile_pool(name="small", bufs=1) as spool,
            tc.tile_pool(name="ps1", bufs=4, space="PSUM") as ps1,
            tc.tile_pool(name="pst", bufs=1, space="PSUM") as pst,
            tc.tile_pool(name="ps2", bufs=2, space="PSUM") as ps2,
            tc.tile_pool(name="ps3", bufs=1, space="PSUM") as ps3,
        ):
            # ---- constants ----
            id128b = cpool.tile([128, 128], BF16)
            masks.make_identity(nc, id128b[:])

            # ---- input DMAs, routed for earliest first-GEMM start ----
            # sync + scalar are the two hardware-DMA engines; gpsimd's
            # software queue only carries the tiny constants.  The first
            # column group's X arrives as ten small [128,512] tiles so the
            # PE starts ~9us in instead of waiting on 512KB tiles.
            wsb = cpool.tile([128, NCH * 66], BF16)
            nc.sync.dma_start(wsb[:], WSW[:])
            xts0a = []
            for k in range(NCH):
                xt = xpool.tile([128, 512], BF16, tag=f"xa{k}")
                nc.sync.dma_start(xt[:], X[k * 128 : (k + 1) * 128, 0:512])
                xts0a.append(xt)

            occt = cpool.tile([128, NCK * NI], BF16)
            nc.scalar.dma_start(occt[:], OCCT[:])
            xts0b = []
            for k in range(NCH):
                xt = xpool.tile([128, 1536], BF16, tag=f"xb{k}")
                nc.scalar.dma_start(xt[:], X[k * 128 : (k + 1) * 128, 512:2048])
                xts0b.append(xt)

            xts1 = []
            for k in range(NCH):
                xt = xpool.tile([128, 2048], BF16, tag=f"xc{k}")
                nc.sync.dma_start(
                    xt[:], X[k * 128 : (k + 1) * 128, 2048:4096]
                )
                xts1.append(xt)

            wc3sb = cpool.tile([3, 66], BF16)
            nc.gpsimd.dma_start(wc3sb[:], WC3[:])
            c3sb = cpool.tile([3, NCELL], BF16)
            nc.gpsimd.dma_start(c3sb[:], C3[:])
            qsb = cpool.tile([DK, NCAPS], BF16)
            nc.gpsimd.dma_start(qsb[:], QTB[:])
            wc2sb = cpool.tile([128, 2 * NCK], BF16)
            nc.gpsimd.dma_start(wc2sb[:], WC2[:])

            def xsrc(j, k):
                if j == 0:
                    return xts0a[k][:, 0:512]
                if j < 4:
                    return xts0b[k][:, (j - 1) * 512 : j * 512]
                return xts1[k][:, (j - 4) * 512 : (j - 3) * 512]

            def body():
                # A-tile strip: [e | e*vl | wcorr | 1] per 128-cell chunk;
                # the wcorr/ones columns are filled once from WC2.
                atall = cpool.tile([128, NCK * 40], BF16)
                atv = atall[:].rearrange("p (c f) -> p c f", f=40)
                wc2v = wc2sb[:].rearrange("p (c two) -> p c two", two=2)
                nc.vector.tensor_copy(atv[:, :, 38:40], wc2v)

                # ---- main pipeline: per 512-cell column group j ----
                m1 = m1pool.tile([66, NCELL], BF16)
                pst_all = pst.tile([128, 2 * NCK], BF16)
                va = spool.tile([128, 2 * NCK], F32)
                sg = spool.tile([128, NCK], F32)
                sgw = spool.tile([128, NCK], F32)
                aml = spool.tile([128, NCK], F32)
                psum3 = ps3.tile([NI, 40], F32)
                vav = va[:].rearrange("p (c two) -> p c two", two=2)
                for j in range(8):
                    psum_j = ps1.tile([66, 512], F32, tag="ps1")
                    for k in range(NCH):
                        nc.tensor.matmul(
                            psum_j[:],
                            wsb[:, k * 66 : (k + 1) * 66],
                            xsrc(j, k),
                            start=(k == 0),
                            stop=False,
                        )
                    nc.tensor.matmul(
                        psum_j[:],
                        wc3sb[:],
                        c3sb[:, j * 512 : (j + 1) * 512],
                        start=False,
                        stop=True,
                    )
                    nc.scalar.copy(m1[:, j * 512 : (j + 1) * 512], psum_j[:])
                    # transpose [vl; alogit] for this group's 4 chunks
                    for s in range(4):
                        jj = 4 * j + s
                        cs = slice(jj * 128, (jj + 1) * 128)
                        nc.tensor.matmul(
                            pst_all[:, 2 * jj : 2 * jj + 2],
                            m1[64:66, cs],
                            id128b[64:66, 64:66],
                            is_transpose=True,
                        )
                    nc.vector.tensor_copy(
                        va[:, 8 * j : 8 * j + 8], pst_all[:, 8 * j : 8 * j + 8]
                    )
                    # am_l = ln(sigmoid(z)+eps) = ln(1+eps+eps*e^-z) - ln(1+e^-z)
                    # batched over this group's 4 chunks; exp/ln only so the
                    # ACT engine stays on one function-table set
                    js = slice(4 * j, 4 * j + 4)
                    nc.scalar.activation(
                        sg[:, js], vav[:, js, 1],
                        mybir.ActivationFunctionType.Exp, scale=-1.0,
                    )
                    nc.vector.tensor_scalar(
                        sg[:, js], sg[:, js], 1.0, None, op0=mybir.AluOpType.add
                    )
                    nc.vector.tensor_scalar(
                        sgw[:, js], sg[:, js], EPS, 1.0,
                        op0=mybir.AluOpType.mult, op1=mybir.AluOpType.add,
                    )
                    nc.scalar.activation(
                        sg[:, js], sg[:, js], mybir.ActivationFunctionType.Ln
                    )
                    nc.scalar.activation(
                        sgw[:, js], sgw[:, js], mybir.ActivationFunctionType.Ln
                    )
                    nc.vector.tensor_tensor(
                        aml[:, js], sgw[:, js], sg[:, js],
                        op=mybir.AluOpType.subtract,
                    )

                    # scores + A-tiles + accumulation for this group's 4 chunks
                    for s in range(4):
                        jj = 4 * j + s
                        cs = slice(jj * 128, (jj + 1) * 128)
                        psum2 = ps2.tile([128, NCAPS], F32, tag="ps2")
                        nc.tensor.matmul(psum2[:], m1[0:64, cs], qsb[:])
                        nc.scalar.activation(
                            atall[:, 40 * jj : 40 * jj + NCAPS],
                            psum2[:],
                            mybir.ActivationFunctionType.Exp,
                            bias=aml[:, jj : jj + 1],
                        )
                        nc.vector.tensor_scalar(
                            atall[:, 40 * jj + NCAPS : 40 * jj + 2 * NCAPS],
                            atall[:, 40 * jj : 40 * jj + NCAPS],
                            va[:, 2 * jj : 2 * jj + 1],
                            None,
                            op0=mybir.AluOpType.mult,
                        )
                        nc.tensor.matmul(
                            psum3[:],
                            occt[:, jj * NI : (jj + 1) * NI],
                            atall[:, 40 * jj : 40 * (jj + 1)],
                            start=(jj == 0),
                            stop=(jj == NCK - 1),
                        )

                # ---- finalize: sigmoid(num/den + corr/n), PSUM read direct ----
                t1 = spool.tile([NI, NCAPS], F32)
                t2n = spool.tile([NI, 1], F32)
                rc1 = spool.tile([NI, NCAPS], F32)
                rc2 = spool.tile([NI, 1], F32)
                nden = spool.tile([NI, 1], F32)
                ones1 = spool.tile([NI, 1], F32)
                nc.gpsimd.memset(ones1[:], 1.0)
                nc.vector.reciprocal(rc1[:], psum3[:, 0:NCAPS])
                nc.vector.tensor_tensor(
                    t1[:], psum3[:, NCAPS : 2 * NCAPS], rc1[:],
                    op=mybir.AluOpType.mult,
                )
                nc.vector.tensor_scalar(
                    nden[:], psum3[:, 39:40], -1.0, None, op0=mybir.AluOpType.mult
                )
                nc.vector.reciprocal(rc2[:], nden[:])
                nc.vector.tensor_tensor(
                    t2n[:], psum3[:, 38:39], rc2[:], op=mybir.AluOpType.mult
                )
                # sigmoid(L) = exp(-ln(1+exp(-L))) with only exp/ln;
                # L = t1 - t2n folds into the first exp's scale+bias.
                osb = spool.tile([NI, NCAPS], F32)
                nc.scalar.activation(
                    osb[:], t1[:], mybir.ActivationFunctionType.Exp,
                    scale=-1.0, bias=t2n[:],
                )
                nc.scalar.activation(
                    osb[:], osb[:], mybir.ActivationFunctionType.Ln,
                    bias=ones1[:],
                )
                nc.scalar.activation(
                    osb[:], osb[:], mybir.ActivationFunctionType.Exp, scale=-1.0
                )
                nc.scalar.dma_start(OUT[:], osb[:])

            if loop_n == 1:
                body()
            else:
                with tc.For_i(0, loop_n, 1):
                    body()

    nc.compile()
    _CACHE[key] = nc
    return nc


def _fold_weights(Wp, bp, Wa, ba, Q, Wk, bk, Wv, bv, Wl, bl):
    f = lambda t: np.asarray(t, np.float64)
    Wp, bp, Wa, ba, Q, Wk, bk, Wv, bv, Wl, bl = map(
        f, (Wp, bp, Wa, ba, Q, Wk, bk, Wv, bv, Wl, bl)
    )
    wl = Wl[:, 0]
    WK = Wp.T @ Wk[:256]
    wvl_cap = Wv[:256] @ wl
    a, b = Wv[256] @ wl, Wv[257] @ wl

    W_all = np.zeros((CIN + 3, 66), np.float64)
    W_all[:CIN, :64] = WK
    W_all[:CIN, 64] = Wp.T @ wvl_cap
    W_all[:CIN, 65] = Wa[0]
    W_all[CIN + 0, :64] = Wk[256] / 64.0
    W_all[CIN + 1, :64] = Wk[257] / 64.0
    W_all[CIN + 2, :64] = bp @ Wk[:256] + bk
    W_all[CIN + 0, 64] = a / 64.0
    W_all[CIN + 1, 64] = b / 64.0
    W_all[CIN + 2, 64] = bp @ wvl_cap + bv @ wl
    W_all[CIN + 2, 65] = ba[0]

    c = np.arange(NCELL)
    y64 = (c // 64) / 64.0
    x64 = (c % 64) / 64.0
    wcorr = -(a * y64 + b * x64 - bl[0])
    WC2 = np.empty((128, 2 * NCK), np.float64)
    WC2[:, 0::2] = wcorr.reshape(NCK, 128).T
    WC2[:, 1::2] = 1.0

    bf = ml_dtypes.bfloat16
    # SBUF layout: [128, 10*66] with channel-chunk k at columns 66k:66(k+1)
    WSW = np.ascontiguousarray(
        W_all[:CIN].reshape(NCH, 128, 66).transpose(1, 0, 2).reshape(128, NCH * 66)
    ).astype(bf)
    return (
        WSW,
        W_all[CIN:].astype(bf),
        (Q.T / 8.0).astype(bf),
        WC2.astype(bf),
    )


def _make_occt(point_lists):
    """[128 cells-in-chunk, chunk*32+instance] occupancy, per image."""
    bf = ml_dtypes.bfloat16
    pts = np.asarray(point_lists).astype(np.int64)  # [B, NI, 2, NPTS]
    ds = pts // 16
    keys = ds[:, :, 0] * 64 + ds[:, :, 1]  # [B, NI, NPTS]
    occ = np.zeros((B, NI, NCELL), np.float32)
    bi = np.arange(B)[:, None, None]
    ii = np.arange(NI)[None, :, None]
    occ[bi, ii, keys] = 1.0
    occt = np.ascontiguousarray(
        occ.reshape(B, NI, NCK, 128).transpose(0, 3, 2, 1).reshape(B, 128, NCK * NI)
    ).astype(bf)
    return occt


def _make_in_maps(
    feature_output, Wp, bp, Wa, ba, Q, Wk, bk, Wv, bv, Wl, bl, point_lists
):
    WSW, WC3, QTB, WC2 = _fold_weights(Wp, bp, Wa, ba, Q, Wk, bk, Wv, bv, Wl, bl)

    bf = ml_dtypes.bfloat16
    c = np.arange(NCELL)
    C3v = np.stack([c // 64, c % 64, np.ones(NCELL)]).astype(bf)

    fo = np.asarray(feature_output, np.float32).reshape(B, CIN, NCELL).astype(bf)
    occt = _make_occt(point_lists)

    return [
        {
            "X": fo[i],
            "WSW": WSW,
            "WC3": WC3,
            "C3": C3v,
            "QTB": QTB,
            "WC2": WC2,
            "OCCT": occt[i],
        }
        for i in range(B)
    ]


def kernel(
    feature_output, Wp, bp, Wa, ba, Q, Wk, bk, Wv, bv, Wl, bl, point_lists
):
    nc = _build_nc()
    in_maps = _make_in_maps(
        feature_output, Wp, bp, Wa, ba, Q, Wk, bk, Wv, bv, Wl, bl, point_lists
    )
    res = run_bass_kernel_spmd(nc, in_maps, core_ids=list(range(B)))
    return np.stack([res.results[i]["OUT"] for i in range(B)]).astype(np.float32)


# revision 14
# speedup vs baseline: 1.1832x; 1.1832x over previous
"""Capsule-routing kernel for Trainium2, data-parallel over batch (8 cores).

Math: the reference's per-instance routing (unique -> gather -> attention)
is reformulated as a dense masked softmax over the 64x64 cell grid:
  - all per-cell quantities (attention keys, value-scalar, activation logit)
    come from one fused per-image GEMM,
  - the relative-position encoding's mean term cancels in the softmax and
    reduces to a rank-1 correction computed from per-instance occupancy sums,
  - per-instance dedup of points is an occupancy bitmap over cells
    (host-precomputed from the integer point lists, like the folded weights),
  - all 32 instances reduce in a single accumulated PE matmul against the
    occupancy mask.

v3: everything in bf16 (validated max rel err ~1.1e-3 vs the 2e-2 gate):
halves the X HBM traffic and runs the PE at 1 cycle/row instead of
f32r's 2.  The whole bf16 X (10.5 MB) is prefetched into SBUF up front
as 20 [128, 2048] tiles so DMA never stalls on buffering; weights and
the occupancy bitmap are pre-swizzled on the host into their SBUF
layouts so every input is one contiguous DMA dispatch.  The A-tiles
live in one persistent SBUF strip whose positional-correction columns
are filled once, and the final sigmoid reads its PSUM accumulator
directly.
"""
import sys

sys.path.insert(0, "/opt/trn_rl_repo")

import numpy as np
import ml_dtypes

import concourse.bacc as bacc
import concourse.mybir as mybir
from concourse import masks, tile
from concourse.bass_utils import run_bass_kernel_spmd

F32 = mybir.dt.float32
BF16 = mybir.dt.bfloat16

B = 8
CIN = 1280
NCELL = 4096  # 64x64 feature grid
NCAPS = 19
NI = 32  # instances per image
NPTS = 256  # points per instance
DK = 64
EPS = 1e-6
NCH = 10  # channel chunks of 128
NCK = 32  # 128-cell chunks

_CACHE = {}

# Force every activation onto the one table set that covers exp/ln/copy so
# the ACT engine never reloads its function tables mid-kernel.
_ONE_SET = "natural_log_exp_and_others"
_orig_get_tables = None


def _patched_tables(arch):
    full = _orig_get_tables(arch)
    return {
        name: (funcs if name == _ONE_SET else set())
        for name, funcs in full.items()
    }


def _install_act_table_patch():
    global _orig_get_tables
    if _orig_get_tables is None:
        _orig_get_tables = bacc.get_activation_tables
        bacc.get_activation_tables = _patched_tables


def _build_nc(dbg=False, loop_n=1, mode="full"):
    key = ("nc", dbg, loop_n, mode)
    if key in _CACHE:
        return _CACHE[key]

    _install_act_table_patch()
    nc = bacc.Bacc(None, target_bir_lowering=False, debug=False)

    X = nc.dram_tensor("X", [CIN, NCELL], BF16, kind="ExternalInput")
    WSW = nc.dram_tensor("WSW", [128, NCH * 66], BF16, kind="ExternalInput")
    WC3 = nc.dram_tensor("WC3", [3, 66], BF16, kind="ExternalInput")
    C3 = nc.dram_tensor("C3", [3, NCELL], BF16, kind="ExternalInput")
    QTB = nc.dram_tensor("QTB", [66, NCAPS + 2], BF16, kind="ExternalInput")
    WC2 = nc.dram_tensor("WC2", [128, 2 * NCK], BF16, kind="ExternalInput")
    OCCT = nc.dram_tensor("OCCT", [128, NCK * NI], BF16, kind="ExternalInput")
    OUT = nc.dram_tensor("OUT", [NI, NCAPS], F32, kind="ExternalOutput")

    with tile.TileContext(nc) as tc:
        with (
            tc.tile_pool(name="const", bufs=1) as cpool,
            tc.tile_pool(name="xp", bufs=1) as xpool,
            tc.tile_pool(name="m1", bufs=1) as m1pool,
            tc.tile_pool(name="small", bufs=1) as spool,
            tc.tile_pool(name="ps1", bufs=5, space="PSUM") as ps1,
            tc.tile_pool(name="ps2", bufs=2, space="PSUM") as ps2,
            tc.tile_pool(name="ps3", bufs=1, space="PSUM") as ps3,
        ):
            # ---- input DMAs, all X on sync's hardware rings in strict
            # consumption order (early-needed tiles must not share wire
            # bandwidth with late-needed ones).  The first column group
            # arrives as ten small [128,512] tiles so the PE starts ~9us
            # in instead of waiting on 512KB tiles.
            wsb = cpool.tile([128, NCH * 66], BF16)
            nc.sync.dma_start(wsb[:], WSW[:])
            xts0a = []
            for k in range(NCH):
                xt = xpool.tile([128, 512], BF16, tag=f"xa{k}")
                nc.sync.dma_start(xt[:], X[k * 128 : (k + 1) * 128, 0:512])
                xts0a.append(xt)

            occt = cpool.tile([128, NCK * NI], BF16)
            nc.sync.dma_start(occt[:], OCCT[:])
            xts0b = []
            for k in range(NCH):
                xt = xpool.tile([128, 1536], BF16, tag=f"xb{k}")
                nc.sync.dma_start(xt[:], X[k * 128 : (k + 1) * 128, 512:2048])
                xts0b.append(xt)

            xts1 = []
            for k in range(NCH):
                xt = xpool.tile([128, 2048], BF16, tag=f"xc{k}")
                nc.sync.dma_start(
                    xt[:], X[k * 128 : (k + 1) * 128, 2048:4096]
                )
                xts1.append(xt)

            wc3sb = cpool.tile([3, 66], BF16)
            nc.gpsimd.dma_start(wc3sb[:], WC3[:])
            c3sb = cpool.tile([3, NCELL], BF16)
            nc.gpsimd.dma_start(c3sb[:], C3[:])
            qsb = cpool.tile([66, NCAPS + 2], BF16)
            nc.gpsimd.dma_start(qsb[:], QTB[:])
            wc2sb = cpool.tile([128, 2 * NCK], BF16)
            nc.gpsimd.dma_start(wc2sb[:], WC2[:])

            def xsrc(j, k):
                if j == 0:
                    return xts0a[k][:, 0:512]
                if j < 4:
                    return xts0b[k][:, (j - 1) * 512 : j * 512]
                return xts1[k][:, (j - 4) * 512 : (j - 3) * 512]

            def body():
                # A-tile strip: [e | e*vl | wcorr | 1] per 128-cell chunk;
                # the wcorr/ones columns are filled once from WC2.
                atall = cpool.tile([128, NCK * 40], BF16)
                atv = atall[:].rearrange("p (c f) -> p c f", f=40)
                wc2v = wc2sb[:].rearrange("p (c two) -> p c two", two=2)
                nc.vector.tensor_copy(atv[:, :, 38:40], wc2v)

                # ---- main pipeline: per 512-cell column group j ----
                m1 = m1pool.tile([66, NCELL], BF16)
                va = spool.tile([128, 2 * NCK], F32)
                sg = spool.tile([128, NCK], F32)
                sgw = spool.tile([128, NCK], F32)
                aml = spool.tile([128, NCK], F32)
                psum3 = ps3.tile([NI, 40], F32)
                vav = va[:].rearrange("p (c two) -> p c two", two=2)
                for j in range(8):
                    psum_j = ps1.tile([66, 512], F32, tag="ps1")
                    for k in range(NCH):
                        nc.tensor.matmul(
                            psum_j[:],
                            wsb[:, k * 66 : (k + 1) * 66],
                            xsrc(j, k),
                            start=(k == 0),
                            stop=False,
                        )
                    nc.tensor.matmul(
                        psum_j[:],
                        wc3sb[:],
                        c3sb[:, j * 512 : (j + 1) * 512],
                        start=False,
                        stop=True,
                    )
                    nc.scalar.copy(m1[:, j * 512 : (j + 1) * 512], psum_j[:])
                    # scores for this group's 4 chunks; qsb's two extra
                    # identity columns make the matmul also emit [vl; alogit]
                    # transposed into psum columns 19:21 for free
                    psum2 = ps2.tile([128, 4 * 21], F32, tag="ps2")
                    for s in range(4):
                        jj = 4 * j + s
                        cs = slice(jj * 128, (jj + 1) * 128)
                        nc.tensor.matmul(
                            psum2[:, 21 * s : 21 * (s + 1)], m1[0:66, cs], qsb[:]
                        )
                    p2v = psum2[:].rearrange("p (c f) -> p c f", f=21)
                    nc.vector.tensor_copy(vav[:, 4 * j : 4 * j + 4, :], p2v[:, :, 19:21])
                    # am_l = ln(sigmoid(z)+eps) = ln(1+eps+eps*e^-z) - ln(1+e^-z)
                    # batched over this group's 4 chunks; exp/ln only so the
                    # ACT engine stays on one function-table set
                    js = slice(4 * j, 4 * j + 4)
                    nc.scalar.activation(
                        sg[:, js], vav[:, js, 1],
                        mybir.ActivationFunctionType.Exp, scale=-1.0,
                    )
                    nc.vector.tensor_scalar(
                        sg[:, js], sg[:, js], 1.0, None, op0=mybir.AluOpType.add
                    )
                    nc.vector.tensor_scalar(
                        sgw[:, js], sg[:, js], EPS, 1.0,
                        op0=mybir.AluOpType.mult, op1=mybir.AluOpType.add,
                    )
                    nc.scalar.activation(
                        sg[:, js], sg[:, js], mybir.ActivationFunctionType.Ln
                    )
                    nc.scalar.activation(
                        sgw[:, js], sgw[:, js], mybir.ActivationFunctionType.Ln
                    )
                    nc.vector.tensor_tensor(
                        aml[:, js], sgw[:, js], sg[:, js],
                        op=mybir.AluOpType.subtract,
                    )

                    # A-tiles + accumulation for this group's 4 chunks
                    for s in range(4):
                        jj = 4 * j + s
                        nc.scalar.activation(
                            atall[:, 40 * jj : 40 * jj + NCAPS],
                            psum2[:, 21 * s : 21 * s + NCAPS],
                            mybir.ActivationFunctionType.Exp,
                            bias=aml[:, jj : jj + 1],
                        )
                        nc.vector.tensor_scalar(
                            atall[:, 40 * jj + NCAPS : 40 * jj + 2 * NCAPS],
                            atall[:, 40 * jj : 40 * jj + NCAPS],
                            va[:, 2 * jj : 2 * jj + 1],
                            None,
                            op0=mybir.AluOpType.mult,
                        )
                        nc.tensor.matmul(
                            psum3[:],
                            occt[:, jj * NI : (jj + 1) * NI],
                            atall[:, 40 * jj : 40 * (jj + 1)],
                            start=(jj == 0),
                            stop=(jj == NCK - 1),
                        )

                # ---- finalize: sigmoid(num/den + corr/n), PSUM read direct ----
                t1 = spool.tile([NI, NCAPS], F32)
                t2n = spool.tile([NI, 1], F32)
                rc1 = spool.tile([NI, NCAPS], F32)
                rc2 = spool.tile([NI, 1], F32)
                nden = spool.tile([NI, 1], F32)
                ones1 = spool.tile([NI, 1], F32)
                nc.gpsimd.memset(ones1[:], 1.0)
                nc.vector.reciprocal(rc1[:], psum3[:, 0:NCAPS])
                nc.vector.tensor_tensor(
                    t1[:], psum3[:, NCAPS : 2 * NCAPS], rc1[:],
                    op=mybir.AluOpType.mult,
                )
                nc.vector.tensor_scalar(
                    nden[:], psum3[:, 39:40], -1.0, None, op0=mybir.AluOpType.mult
                )
                nc.vector.reciprocal(rc2[:], nden[:])
                nc.vector.tensor_tensor(
                    t2n[:], psum3[:, 38:39], rc2[:], op=mybir.AluOpType.mult
                )
                # sigmoid(L) = exp(-ln(1+exp(-L))) with only exp/ln;
                # L = t1 - t2n folds into the first exp's scale+bias.
                osb = spool.tile([NI, NCAPS], F32)
                nc.scalar.activation(
                    osb[:], t1[:], mybir.ActivationFunctionType.Exp,
                    scale=-1.0, bias=t2n[:],
                )
                nc.scalar.activation(
                    osb[:], osb[:], mybir.ActivationFunctionType.Ln,
                    bias=ones1[:],
                )
                nc.scalar.activation(
                    osb[:], osb[:], mybir.ActivationFunctionType.Exp, scale=-1.0
                )
                nc.scalar.dma_start(OUT[:], osb[:])

            if loop_n == 1:
                body()
            else:
                with tc.For_i(0, loop_n, 1):
                    body()

    nc.compile()
    _CACHE[key] = nc
    return nc


def _fold_weights(Wp, bp, Wa, ba, Q, Wk, bk, Wv, bv, Wl, bl):
    f = lambda t: np.asarray(t, np.float64)
    Wp, bp, Wa, ba, Q, Wk, bk, Wv, bv, Wl, bl = map(
        f, (Wp, bp, Wa, ba, Q, Wk, bk, Wv, bv, Wl, bl)
    )
    wl = Wl[:, 0]
    WK = Wp.T @ Wk[:256]
    wvl_cap = Wv[:256] @ wl
    a, b = Wv[256] @ wl, Wv[257] @ wl

    W_all = np.zeros((CIN + 3, 66), np.float64)
    W_all[:CIN, :64] = WK
    W_all[:CIN, 64] = Wp.T @ wvl_cap
    W_all[:CIN, 65] = Wa[0]
    W_all[CIN + 0, :64] = Wk[256] / 64.0
    W_all[CIN + 1, :64] = Wk[257] / 64.0
    W_all[CIN + 2, :64] = bp @ Wk[:256] + bk
    W_all[CIN + 0, 64] = a / 64.0
    W_all[CIN + 1, 64] = b / 64.0
    W_all[CIN + 2, 64] = bp @ wvl_cap + bv @ wl
    W_all[CIN + 2, 65] = ba[0]

    c = np.arange(NCELL)
    y64 = (c // 64) / 64.0
    x64 = (c % 64) / 64.0
    wcorr = -(a * y64 + b * x64 - bl[0])
    WC2 = np.empty((128, 2 * NCK), np.float64)
    WC2[:, 0::2] = wcorr.reshape(NCK, 128).T
    WC2[:, 1::2] = 1.0

    bf = ml_dtypes.bfloat16
    # SBUF layout: [128, 10*66] with channel-chunk k at columns 66k:66(k+1)
    WSW = np.ascontiguousarray(
        W_all[:CIN].reshape(NCH, 128, 66).transpose(1, 0, 2).reshape(128, NCH * 66)
    ).astype(bf)
    QTB = np.zeros((66, NCAPS + 2), np.float64)
    QTB[:DK, :NCAPS] = Q.T / 8.0
    QTB[64, NCAPS] = 1.0
    QTB[65, NCAPS + 1] = 1.0
    return (
        WSW,
        W_all[CIN:].astype(bf),
        QTB.astype(bf),
        WC2.astype(bf),
    )


def _make_occt(point_lists):
    """[128 cells-in-chunk, chunk*32+instance] occupancy, per image."""
    bf = ml_dtypes.bfloat16
    pts = np.asarray(point_lists).astype(np.int64)  # [B, NI, 2, NPTS]
    ds = pts // 16
    keys = ds[:, :, 0] * 64 + ds[:, :, 1]  # [B, NI, NPTS]
    occ = np.zeros((B, NI, NCELL), np.float32)
    bi = np.arange(B)[:, None, None]
    ii = np.arange(NI)[None, :, None]
    occ[bi, ii, keys] = 1.0
    occt = np.ascontiguousarray(
        occ.reshape(B, NI, NCK, 128).transpose(0, 3, 2, 1).reshape(B, 128, NCK * NI)
    ).astype(bf)
    return occt


def _make_in_maps(
    feature_output, Wp, bp, Wa, ba, Q, Wk, bk, Wv, bv, Wl, bl, point_lists
):
    WSW, WC3, QTB, WC2 = _fold_weights(Wp, bp, Wa, ba, Q, Wk, bk, Wv, bv, Wl, bl)

    bf = ml_dtypes.bfloat16
    c = np.arange(NCELL)
    C3v = np.stack([c // 64, c % 64, np.ones(NCELL)]).astype(bf)

    fo = np.asarray(feature_output, np.float32).reshape(B, CIN, NCELL).astype(bf)
    occt = _make_occt(point_lists)

    return [
        {
            "X": fo[i],
            "WSW": WSW,
            "WC3": WC3,
            "C3": C3v,
            "QTB": QTB,
            "WC2": WC2,
            "OCCT": occt[i],
        }
        for i in range(B)
    ]


def kernel(
    feature_output, Wp, bp, Wa, ba, Q, Wk, bk, Wv, bv, Wl, bl, point_lists
):
    nc = _build_nc()
    in_maps = _make_in_maps(
        feature_output, Wp, bp, Wa, ba, Q, Wk, bk, Wv, bv, Wl, bl, point_lists
    )
    res = run_bass_kernel_spmd(nc, in_maps, core_ids=list(range(B)))
    return np.stack([res.results[i]["OUT"] for i in range(B)]).astype(np.float32)


# revision 19
# speedup vs baseline: 1.2772x; 1.0794x over previous
"""Capsule-routing kernel for Trainium2, data-parallel over batch (8 cores).

Math: the reference's per-instance routing (unique -> gather -> attention)
is reformulated as a dense masked softmax over the 64x64 cell grid:
  - all per-cell quantities (attention keys, value-scalar, activation logit)
    come from one fused per-image GEMM,
  - the relative-position encoding's mean term cancels in the softmax and
    reduces to a rank-1 correction computed from per-instance occupancy sums,
  - per-instance dedup of points is an occupancy bitmap over cells
    (host-precomputed from the integer point lists, like the folded weights),
  - all 32 instances reduce in a single accumulated PE matmul against the
    occupancy mask.

v3: everything in bf16 (validated max rel err ~1.1e-3 vs the 2e-2 gate):
halves the X HBM traffic and runs the PE at 1 cycle/row instead of
f32r's 2.  The whole bf16 X (10.5 MB) is prefetched into SBUF up front
as 20 [128, 2048] tiles so DMA never stalls on buffering; weights and
the occupancy bitmap are pre-swizzled on the host into their SBUF
layouts so every input is one contiguous DMA dispatch.  The A-tiles
live in one persistent SBUF strip whose positional-correction columns
are filled once, and the final sigmoid reads its PSUM accumulator
directly.
"""
import sys

sys.path.insert(0, "/opt/trn_rl_repo")

import numpy as np
import ml_dtypes

import concourse.bacc as bacc
import concourse.mybir as mybir
from concourse import masks, tile
from concourse.bass_utils import run_bass_kernel_spmd

F32 = mybir.dt.float32
BF16 = mybir.dt.bfloat16
FP8 = mybir.dt.float8e4

B = 8
CIN = 1280
NCELL = 4096  # 64x64 feature grid
NCAPS = 19
NI = 32  # instances per image
NPTS = 256  # points per instance
DK = 64
EPS = 1e-6
NCH = 10  # channel chunks of 128
NCK = 32  # 128-cell chunks

_CACHE = {}

# Force every activation onto the one table set that covers exp/ln/copy so
# the ACT engine never reloads its function tables mid-kernel.
_ONE_SET = "natural_log_exp_and_others"
_orig_get_tables = None


def _patched_tables(arch):
    full = _orig_get_tables(arch)
    return {
        name: (funcs if name == _ONE_SET else set())
        for name, funcs in full.items()
    }


def _install_act_table_patch():
    global _orig_get_tables
    if _orig_get_tables is None:
        _orig_get_tables = bacc.get_activation_tables
        bacc.get_activation_tables = _patched_tables


def _build_nc(dbg=False, loop_n=1, mode="full"):
    key = ("nc", dbg, loop_n, mode)
    if key in _CACHE:
        return _CACHE[key]

    _install_act_table_patch()
    nc = bacc.Bacc(None, target_bir_lowering=False, debug=False)

    # X/WSW in fp8 e4m3 pair layout for DoubleRow matmuls:
    # X rows p*128+part, cols sub*NCELL+cell (pair p = channel chunks 2p,2p+1).
    # WSW folds Q into the channel weights (x64 scaled into fp8 range):
    # 32 output cols = [19 scores | vl | alog | 11 pad].
    X = nc.dram_tensor("X", [5 * 128, 2 * NCELL], FP8, kind="ExternalInput")
    WSW = nc.dram_tensor("WSW", [128, 5 * 2 * 32], FP8, kind="ExternalInput")
    C3M1 = nc.dram_tensor("C3M1", [21, NCELL], BF16, kind="ExternalInput")
    WC2 = nc.dram_tensor("WC2", [128, 2 * NCK], BF16, kind="ExternalInput")
    OCCT = nc.dram_tensor("OCCT", [128, NCK * NI], BF16, kind="ExternalInput")
    OUT = nc.dram_tensor("OUT", [NI, NCAPS], F32, kind="ExternalOutput")

    with tile.TileContext(nc) as tc:
        with (
            tc.tile_pool(name="const", bufs=1) as cpool,
            tc.tile_pool(name="xp", bufs=1) as xpool,
            tc.tile_pool(name="m1", bufs=1) as m1pool,
            tc.tile_pool(name="small", bufs=1) as spool,
            tc.tile_pool(name="ps1", bufs=5, space="PSUM") as ps1,
            tc.tile_pool(name="ps2", bufs=2, space="PSUM") as ps2,
            tc.tile_pool(name="ps3", bufs=1, space="PSUM") as ps3,
        ):
            # ---- input DMAs, all X on sync's hardware rings in strict
            # consumption order (early-needed tiles must not share wire
            # bandwidth with late-needed ones).  The first column group
            # arrives as ten small [128,512] tiles so the PE starts ~9us
            # in instead of waiting on 512KB tiles.
            wsb = cpool.tile([128, 5 * 2 * 32], FP8)
            nc.sync.dma_start(wsb[:], WSW[:])
            c3sb = cpool.tile([21, NCELL], BF16)
            nc.sync.dma_start(c3sb[:], C3M1[:])
            xv = X[:].rearrange("(p q) (s c) -> p q s c", q=128, s=2)
            xts0a = []
            for p in range(5):
                xt = xpool.tile([128, 2 * 512], FP8, tag=f"xa{p}")
                nc.sync.dma_start(xt[:], xv[p, :, :, 0:512])
                xts0a.append(xt)

            occt = cpool.tile([128, NCK * NI], BF16)
            nc.sync.dma_start(occt[:], OCCT[:])
            xts0b = []
            for p in range(5):
                xt = xpool.tile([128, 2 * 1536], FP8, tag=f"xb{p}")
                nc.sync.dma_start(xt[:], xv[p, :, :, 512:2048])
                xts0b.append(xt)

            xts1 = []
            for p in range(5):
                xt = xpool.tile([128, 2 * 2048], FP8, tag=f"xc{p}")
                nc.sync.dma_start(xt[:], xv[p, :, :, 2048:4096])
                xts1.append(xt)

            wc2sb = cpool.tile([128, 2 * NCK], BF16)
            nc.gpsimd.dma_start(wc2sb[:], WC2[:])

            def xsrc(j, p):
                if j == 0:
                    t, w, jo = xts0a[p], 512, 0
                elif j < 4:
                    t, w, jo = xts0b[p], 1536, j - 1
                else:
                    t, w, jo = xts1[p], 2048, j - 4
                v = t[:].rearrange("q (s c) -> q s c", s=2)
                return v[:, :, jo * 512 : (jo + 1) * 512]

            id32b = cpool.tile([32, 32], BF16)
            masks.make_identity(nc, id32b[:])

            def body():
                # A-tile strip: [e | e*vl | wcorr | 1] per 128-cell chunk;
                # the wcorr/ones columns are filled once from WC2.
                atall = cpool.tile([128, NCK * 40], BF16)
                atv = atall[:].rearrange("p (c f) -> p c f", f=40)
                wc2v = wc2sb[:].rearrange("p (c two) -> p c two", two=2)
                nc.vector.tensor_copy(atv[:, :, 38:40], wc2v)

                # ---- main pipeline: per 512-cell column group j ----
                m1 = m1pool.tile([21, NCELL], BF16)
                va = spool.tile([128, 2 * NCK], F32)
                sg = spool.tile([128, NCK], F32)
                sgw = spool.tile([128, NCK], F32)
                aml = spool.tile([128, NCK], F32)
                psum3 = ps3.tile([NI, 40], F32)
                vav = va[:].rearrange("p (c two) -> p c two", two=2)
                for j in range(8):
                    psum_j = ps1.tile([32, 512], F32, tag="ps1")
                    wv = wsb[:].rearrange("q (p s f) -> q p s f", p=5, s=2)
                    for p in range(5):
                        nc.tensor.matmul(
                            psum_j[:],
                            wv[:, p, :, :],
                            xsrc(j, p),
                            start=(p == 0),
                            stop=(p == 4),
                            perf_mode=mybir.MatmulPerfMode.DoubleRow,
                        )
                    # m1 = psum/64 + const-path contribution (one fused DVE op)
                    nc.vector.scalar_tensor_tensor(
                        m1[:, j * 512 : (j + 1) * 512],
                        psum_j[0:21, :],
                        1.0 / 64.0,
                        c3sb[:, j * 512 : (j + 1) * 512],
                        op0=mybir.AluOpType.mult,
                        op1=mybir.AluOpType.add,
                    )
                    # transpose [scores | vl | alog] for this group's 4 chunks
                    psum2 = ps2.tile([128, 4 * 24], BF16, tag="ps2")
                    for s in range(4):
                        jj = 4 * j + s
                        cs = slice(jj * 128, (jj + 1) * 128)
                        nc.tensor.matmul(
                            psum2[:, 24 * s : 24 * s + 21],
                            m1[0:21, cs],
                            id32b[0:21, 0:21],
                            is_transpose=True,
                        )
                    p2v = psum2[:].rearrange("p (c f) -> p c f", f=24)
                    nc.vector.tensor_copy(vav[:, 4 * j : 4 * j + 4, :], p2v[:, :, 19:21])
                    # am_l = ln(sigmoid(z)+eps) = ln(1+eps+eps*e^-z) - ln(1+e^-z)
                    # batched over this group's 4 chunks; exp/ln only so the
                    # ACT engine stays on one function-table set
                    js = slice(4 * j, 4 * j + 4)
                    nc.scalar.activation(
                        sg[:, js], vav[:, js, 1],
                        mybir.ActivationFunctionType.Exp, scale=-1.0,
                    )
                    nc.vector.tensor_scalar(
                        sg[:, js], sg[:, js], 1.0, None, op0=mybir.AluOpType.add
                    )
                    nc.vector.tensor_scalar(
                        sgw[:, js], sg[:, js], EPS, 1.0,
                        op0=mybir.AluOpType.mult, op1=mybir.AluOpType.add,
                    )
                    nc.scalar.activation(
                        sg[:, js], sg[:, js], mybir.ActivationFunctionType.Ln
                    )
                    nc.scalar.activation(
                        sgw[:, js], sgw[:, js], mybir.ActivationFunctionType.Ln
                    )
                    nc.vector.tensor_tensor(
                        aml[:, js], sgw[:, js], sg[:, js],
                        op=mybir.AluOpType.subtract,
                    )

                    # A-tiles + accumulation for this group's 4 chunks
                    for s in range(4):
                        jj = 4 * j + s
                        nc.scalar.activation(
                            atall[:, 40 * jj : 40 * jj + NCAPS],
                            psum2[:, 24 * s : 24 * s + NCAPS],
                            mybir.ActivationFunctionType.Exp,
                            bias=aml[:, jj : jj + 1],
                        )
                        nc.vector.tensor_scalar(
                            atall[:, 40 * jj + NCAPS : 40 * jj + 2 * NCAPS],
                            atall[:, 40 * jj : 40 * jj + NCAPS],
                            va[:, 2 * jj : 2 * jj + 1],
                            None,
                            op0=mybir.AluOpType.mult,
                        )
                        nc.tensor.matmul(
                            psum3[:],
                            occt[:, jj * NI : (jj + 1) * NI],
                            atall[:, 40 * jj : 40 * (jj + 1)],
                            start=(jj == 0),
                            stop=(jj == NCK - 1),
                        )

                # ---- finalize: sigmoid(num/den + corr/n), PSUM read direct ----
                t1 = spool.tile([NI, NCAPS], F32)
                t2n = spool.tile([NI, 1], F32)
                rc1 = spool.tile([NI, NCAPS], F32)
                rc2 = spool.tile([NI, 1], F32)
                nden = spool.tile([NI, 1], F32)
                ones1 = spool.tile([NI, 1], F32)
                nc.gpsimd.memset(ones1[:], 1.0)
                nc.vector.reciprocal(rc1[:], psum3[:, 0:NCAPS])
                nc.vector.tensor_tensor(
                    t1[:], psum3[:, NCAPS : 2 * NCAPS], rc1[:],
                    op=mybir.AluOpType.mult,
                )
                nc.vector.tensor_scalar(
                    nden[:], psum3[:, 39:40], -1.0, None, op0=mybir.AluOpType.mult
                )
                nc.vector.reciprocal(rc2[:], nden[:])
                nc.vector.tensor_tensor(
                    t2n[:], psum3[:, 38:39], rc2[:], op=mybir.AluOpType.mult
                )
                # sigmoid(L) = exp(-ln(1+exp(-L))) with only exp/ln;
                # L = t1 - t2n folds into the first exp's scale+bias.
                osb = spool.tile([NI, NCAPS], F32)
                nc.scalar.activation(
                    osb[:], t1[:], mybir.ActivationFunctionType.Exp,
                    scale=-1.0, bias=t2n[:],
                )
                nc.scalar.activation(
                    osb[:], osb[:], mybir.ActivationFunctionType.Ln,
                    bias=ones1[:],
                )
                nc.scalar.activation(
                    osb[:], osb[:], mybir.ActivationFunctionType.Exp, scale=-1.0
                )
                nc.scalar.dma_start(OUT[:], osb[:])

            if loop_n == 1:
                body()
            else:
                with tc.For_i(0, loop_n, 1):
                    body()

    nc.compile()
    _CACHE[key] = nc
    return nc


def _fold_weights(Wp, bp, Wa, ba, Q, Wk, bk, Wv, bv, Wl, bl):
    f = lambda t: np.asarray(t, np.float64)
    Wp, bp, Wa, ba, Q, Wk, bk, Wv, bv, Wl, bl = map(
        f, (Wp, bp, Wa, ba, Q, Wk, bk, Wv, bv, Wl, bl)
    )
    wl = Wl[:, 0]
    WK = Wp.T @ Wk[:256]
    wvl_cap = Wv[:256] @ wl
    a, b = Wv[256] @ wl, Wv[257] @ wl

    W_all = np.zeros((CIN + 3, 66), np.float64)
    W_all[:CIN, :64] = WK
    W_all[:CIN, 64] = Wp.T @ wvl_cap
    W_all[:CIN, 65] = Wa[0]
    W_all[CIN + 0, :64] = Wk[256] / 64.0
    W_all[CIN + 1, :64] = Wk[257] / 64.0
    W_all[CIN + 2, :64] = bp @ Wk[:256] + bk
    W_all[CIN + 0, 64] = a / 64.0
    W_all[CIN + 1, 64] = b / 64.0
    W_all[CIN + 2, 64] = bp @ wvl_cap + bv @ wl
    W_all[CIN + 2, 65] = ba[0]

    c = np.arange(NCELL)
    y64 = (c // 64) / 64.0
    x64 = (c % 64) / 64.0
    wcorr = -(a * y64 + b * x64 - bl[0])
    WC2 = np.empty((128, 2 * NCK), np.float64)
    WC2[:, 0::2] = wcorr.reshape(NCK, 128).T
    WC2[:, 1::2] = 1.0

    bf = ml_dtypes.bfloat16
    # Fold Q into the channel weights: 21 useful output cols
    # [19 scores | vl | alog], padded to 32 for the fp8 DoubleRow LDW
    # restriction, scaled x64 into fp8 e4m3's normal range (descaled on chip).
    Wq = np.zeros((CIN + 3, 32))
    Wq[:, 0:NCAPS] = W_all[:, 0:64] @ (Q.T / 8.0)
    Wq[:, NCAPS : NCAPS + 2] = W_all[:, 64:66]
    WSW = np.ascontiguousarray(
        (Wq[:CIN] * 64.0)
        .reshape(5, 2, 128, 32)
        .transpose(2, 0, 1, 3)
        .reshape(128, 5 * 2 * 32)
    ).astype(ml_dtypes.float8_e4m3)
    c = np.arange(NCELL)
    C3v = np.stack([c // 64, c % 64, np.ones(NCELL)])
    C3M1 = (Wq[CIN:, 0:21].T @ C3v).astype(bf)
    return (
        WSW,
        C3M1,
        WC2.astype(bf),
    )


def _make_occt(point_lists):
    """[128 cells-in-chunk, chunk*32+instance] occupancy, per image."""
    bf = ml_dtypes.bfloat16
    pts = np.asarray(point_lists).astype(np.int64)  # [B, NI, 2, NPTS]
    ds = pts // 16
    keys = ds[:, :, 0] * 64 + ds[:, :, 1]  # [B, NI, NPTS]
    occ = np.zeros((B, NI, NCELL), np.float32)
    bi = np.arange(B)[:, None, None]
    ii = np.arange(NI)[None, :, None]
    occ[bi, ii, keys] = 1.0
    occt = np.ascontiguousarray(
        occ.reshape(B, NI, NCK, 128).transpose(0, 3, 2, 1).reshape(B, 128, NCK * NI)
    ).astype(bf)
    return occt


def _make_in_maps(
    feature_output, Wp, bp, Wa, ba, Q, Wk, bk, Wv, bv, Wl, bl, point_lists
):
    WSW, C3M1, WC2 = _fold_weights(Wp, bp, Wa, ba, Q, Wk, bk, Wv, bv, Wl, bl)

    # fp8 pair layout: [5 pairs, 128 part, 2 sub, 4096 cells] -> [640, 8192]
    fo = (
        np.asarray(feature_output, np.float32)
        .reshape(B, 5, 2, 128, NCELL)
        .transpose(0, 1, 3, 2, 4)
        .reshape(B, 5 * 128, 2 * NCELL)
        .astype(ml_dtypes.float8_e4m3)
    )
    occt = _make_occt(point_lists)

    return [
        {
            "X": fo[i],
            "WSW": WSW,
            "C3M1": C3M1,
            "WC2": WC2,
            "OCCT": occt[i],
        }
        for i in range(B)
    ]


def kernel(
    feature_output, Wp, bp, Wa, ba, Q, Wk, bk, Wv, bv, Wl, bl, point_lists
):
    nc = _build_nc()
    in_maps = _make_in_maps(
        feature_output, Wp, bp, Wa, ba, Q, Wk, bk, Wv, bv, Wl, bl, point_lists
    )
    res = run_bass_kernel_spmd(nc, in_maps, core_ids=list(range(B)))
    return np.stack([res.results[i]["OUT"] for i in range(B)]).astype(np.float32)


# revision 20
# speedup vs baseline: 1.4835x; 1.1615x over previous
"""Capsule-routing kernel for Trainium2, data-parallel over batch (8 cores).

Math: the reference's per-instance routing (unique -> gather -> attention)
is reformulated as a dense masked softmax over the 64x64 cell grid:
  - all per-cell quantities (attention keys, value-scalar, activation logit)
    come from one fused per-image GEMM,
  - the relative-position encoding's mean term cancels in the softmax and
    reduces to a rank-1 correction computed from per-instance occupancy sums,
  - per-instance dedup of points is an occupancy bitmap over cells
    (host-precomputed from the integer point lists, like the folded weights),
  - all 32 instances reduce in a single accumulated PE matmul against the
    occupancy mask.

v3: everything in bf16 (validated max rel err ~1.1e-3 vs the 2e-2 gate):
halves the X HBM traffic and runs the PE at 1 cycle/row instead of
f32r's 2.  The whole bf16 X (10.5 MB) is prefetched into SBUF up front
as 20 [128, 2048] tiles so DMA never stalls on buffering; weights and
the occupancy bitmap are pre-swizzled on the host into their SBUF
layouts so every input is one contiguous DMA dispatch.  The A-tiles
live in one persistent SBUF strip whose positional-correction columns
are filled once, and the final sigmoid reads its PSUM accumulator
directly.
"""
import sys

sys.path.insert(0, "/opt/trn_rl_repo")

import numpy as np
import ml_dtypes

import concourse.bacc as bacc
import concourse.mybir as mybir
from concourse import masks, tile
from concourse.bass_utils import run_bass_kernel_spmd

F32 = mybir.dt.float32
BF16 = mybir.dt.bfloat16
FP8 = mybir.dt.float8e4

B = 8
CIN = 1280
NCELL = 4096  # 64x64 feature grid
NCAPS = 19
NI = 32  # instances per image
NPTS = 256  # points per instance
DK = 64
EPS = 1e-6
NCH = 10  # channel chunks of 128
NCK = 32  # 128-cell chunks

_CACHE = {}

# Force every activation onto the one table set that covers exp/ln/copy so
# the ACT engine never reloads its function tables mid-kernel.
_ONE_SET = "natural_log_exp_and_others"
_orig_get_tables = None


def _patched_tables(arch):
    full = _orig_get_tables(arch)
    return {
        name: (funcs if name == _ONE_SET else set())
        for name, funcs in full.items()
    }


def _install_act_table_patch():
    global _orig_get_tables
    if _orig_get_tables is None:
        _orig_get_tables = bacc.get_activation_tables
        bacc.get_activation_tables = _patched_tables


def _build_nc(dbg=False, loop_n=1, mode="full"):
    key = ("nc", dbg, loop_n, mode)
    if key in _CACHE:
        return _CACHE[key]

    _install_act_table_patch()
    nc = bacc.Bacc(None, target_bir_lowering=False, debug=False)

    # X/WSW in fp8 e4m3 pair layout for DoubleRow matmuls:
    # X rows p*128+part, cols sub*NCELL+cell (pair p = channel chunks 2p,2p+1).
    # WSW folds Q into the channel weights (x64 scaled into fp8 range):
    # 32 output cols = [19 scores | vl | alog | 11 pad].
    X = nc.dram_tensor("X", [5 * 128, 2 * NCELL], FP8, kind="ExternalInput")
    WSW = nc.dram_tensor("WSW", [128, 5 * 2 * 32], FP8, kind="ExternalInput")
    C3M1 = nc.dram_tensor("C3M1", [21, NCELL], BF16, kind="ExternalInput")
    WC2 = nc.dram_tensor("WC2", [128, 2 * NCK], BF16, kind="ExternalInput")
    OCCT = nc.dram_tensor("OCCT", [128, NCK * NI], BF16, kind="ExternalInput")
    OUT = nc.dram_tensor("OUT", [NI, NCAPS], F32, kind="ExternalOutput")

    with tile.TileContext(nc) as tc:
        with (
            tc.tile_pool(name="const", bufs=1) as cpool,
            tc.tile_pool(name="xp", bufs=1) as xpool,
            tc.tile_pool(name="m1", bufs=1) as m1pool,
            tc.tile_pool(name="small", bufs=1) as spool,
            tc.tile_pool(name="ps1", bufs=4, space="PSUM") as ps1,
            tc.tile_pool(name="ps2", bufs=3, space="PSUM") as ps2,
            tc.tile_pool(name="ps3", bufs=1, space="PSUM") as ps3,
        ):
            # ---- input DMAs, all X on sync's hardware rings in strict
            # consumption order (early-needed tiles must not share wire
            # bandwidth with late-needed ones).  The first column group
            # arrives as ten small [128,512] tiles so the PE starts ~9us
            # in instead of waiting on 512KB tiles.
            wsb = cpool.tile([128, 5 * 2 * 32], FP8)
            nc.sync.dma_start(wsb[:], WSW[:])
            xv = X[:].rearrange("(p q) (s c) -> p q s c", q=128, s=2)
            xts0a = []
            for p in range(5):
                xt = xpool.tile([128, 2 * 512], FP8, tag=f"xa{p}")
                nc.sync.dma_start(xt[:], xv[p, :, :, 0:512])
                xts0a.append(xt)

            c3sb = cpool.tile([21, NCELL], BF16)
            nc.sync.dma_start(c3sb[:], C3M1[:])
            occt = cpool.tile([128, NCK * NI], BF16)
            nc.sync.dma_start(occt[:], OCCT[:])
            xts0b = []
            for p in range(5):
                xt = xpool.tile([128, 2 * 1536], FP8, tag=f"xb{p}")
                nc.sync.dma_start(xt[:], xv[p, :, :, 512:2048])
                xts0b.append(xt)

            xts1 = []
            for p in range(5):
                xt = xpool.tile([128, 2 * 2048], FP8, tag=f"xc{p}")
                nc.sync.dma_start(xt[:], xv[p, :, :, 2048:4096])
                xts1.append(xt)

            wc2sb = cpool.tile([128, 2 * NCK], BF16)
            nc.gpsimd.dma_start(wc2sb[:], WC2[:])

            def xsrc(j, p):
                if j == 0:
                    t, w, jo = xts0a[p], 512, 0
                elif j < 4:
                    t, w, jo = xts0b[p], 1536, j - 1
                else:
                    t, w, jo = xts1[p], 2048, j - 4
                v = t[:].rearrange("q (s c) -> q s c", s=2)
                return v[:, :, jo * 512 : (jo + 1) * 512]

            id32b = cpool.tile([32, 32], BF16)
            masks.make_identity(nc, id32b[:])

            def body():
                # A-tile strip: [e | e*vl | wcorr | 1] per 128-cell chunk;
                # the wcorr/ones columns are filled once from WC2.
                atall = cpool.tile([128, NCK * 40], BF16)
                atv = atall[:].rearrange("p (c f) -> p c f", f=40)
                wc2v = wc2sb[:].rearrange("p (c two) -> p c two", two=2)
                nc.vector.tensor_copy(atv[:, :, 38:40], wc2v)

                # ---- main pipeline: per 512-cell column group j ----
                m1 = m1pool.tile([21, NCELL], BF16)
                ones128 = spool.tile([128, 1], F32)
                nc.gpsimd.memset(ones128[:], 1.0)
                va = spool.tile([128, 2 * NCK], F32)
                sg = spool.tile([128, NCK], F32)
                sgw = spool.tile([128, NCK], F32)
                aml = spool.tile([128, NCK], F32)
                psum3 = ps3.tile([NI, 40], F32)
                vav = va[:].rearrange("p (c two) -> p c two", two=2)
                for j in range(8):
                    psum_j = ps1.tile([32, 512], F32, tag="ps1")
                    wv = wsb[:].rearrange("q (p s f) -> q p s f", p=5, s=2)
                    for p in range(5):
                        nc.tensor.matmul(
                            psum_j[:],
                            wv[:, p, :, :],
                            xsrc(j, p),
                            start=(p == 0),
                            stop=(p == 4),
                            perf_mode=mybir.MatmulPerfMode.DoubleRow,
                        )
                    # m1 = psum/64 + const-path contribution (one fused DVE op)
                    nc.vector.scalar_tensor_tensor(
                        m1[:, j * 512 : (j + 1) * 512],
                        psum_j[0:21, :],
                        1.0 / 64.0,
                        c3sb[:, j * 512 : (j + 1) * 512],
                        op0=mybir.AluOpType.mult,
                        op1=mybir.AluOpType.add,
                    )
                    # transpose [scores | vl | alog] for this group's 4 chunks
                    psum2 = ps2.tile([128, 4 * 24], BF16, tag="ps2")
                    for s in range(4):
                        jj = 4 * j + s
                        cs = slice(jj * 128, (jj + 1) * 128)
                        nc.tensor.matmul(
                            psum2[:, 24 * s : 24 * s + 21],
                            m1[0:21, cs],
                            id32b[0:21, 0:21],
                            is_transpose=True,
                        )
                    p2v = psum2[:].rearrange("p (c f) -> p c f", f=24)
                    nc.vector.tensor_copy(vav[:, 4 * j : 4 * j + 4, :], p2v[:, :, 19:21])
                    # am_l = ln(sigmoid(z)+eps) = ln(1+eps+eps*e^-z) - ln(1+e^-z)
                    # batched over this group's 4 chunks; exp/ln only so the
                    # ACT engine stays on one function-table set
                    js = slice(4 * j, 4 * j + 4)
                    nc.scalar.activation(
                        sg[:, js], vav[:, js, 1],
                        mybir.ActivationFunctionType.Exp, scale=-1.0,
                    )
                    nc.vector.tensor_scalar(
                        sgw[:, js], sg[:, js], EPS, 1.0 + EPS,
                        op0=mybir.AluOpType.mult, op1=mybir.AluOpType.add,
                    )
                    nc.scalar.activation(
                        sg[:, js], sg[:, js], mybir.ActivationFunctionType.Ln,
                        bias=ones128[:],
                    )
                    nc.scalar.activation(
                        sgw[:, js], sgw[:, js], mybir.ActivationFunctionType.Ln
                    )
                    nc.vector.tensor_tensor(
                        aml[:, js], sgw[:, js], sg[:, js],
                        op=mybir.AluOpType.subtract,
                    )

                    # A-tiles + accumulation for this group's 4 chunks
                    for s in range(4):
                        jj = 4 * j + s
                        nc.scalar.activation(
                            atall[:, 40 * jj : 40 * jj + NCAPS],
                            psum2[:, 24 * s : 24 * s + NCAPS],
                            mybir.ActivationFunctionType.Exp,
                            bias=aml[:, jj : jj + 1],
                        )
                        mult_eng = nc.vector if s % 2 == 0 else nc.gpsimd
                        mult_eng.tensor_scalar(
                            atall[:, 40 * jj + NCAPS : 40 * jj + 2 * NCAPS],
                            atall[:, 40 * jj : 40 * jj + NCAPS],
                            va[:, 2 * jj : 2 * jj + 1],
                            None,
                            op0=mybir.AluOpType.mult,
                        )
                        nc.tensor.matmul(
                            psum3[:],
                            occt[:, jj * NI : (jj + 1) * NI],
                            atall[:, 40 * jj : 40 * (jj + 1)],
                            start=(jj == 0),
                            stop=(jj == NCK - 1),
                        )

                # ---- finalize: sigmoid(num/den + corr/n), PSUM read direct ----
                t1 = spool.tile([NI, NCAPS], F32)
                t2n = spool.tile([NI, 1], F32)
                rc1 = spool.tile([NI, NCAPS], F32)
                rc2 = spool.tile([NI, 1], F32)
                nden = spool.tile([NI, 1], F32)
                nc.vector.reciprocal(rc1[:], psum3[:, 0:NCAPS])
                nc.vector.tensor_tensor(
                    t1[:], psum3[:, NCAPS : 2 * NCAPS], rc1[:],
                    op=mybir.AluOpType.mult,
                )
                nc.vector.tensor_scalar(
                    nden[:], psum3[:, 39:40], -1.0, None, op0=mybir.AluOpType.mult
                )
                nc.vector.reciprocal(rc2[:], nden[:])
                nc.vector.tensor_tensor(
                    t2n[:], psum3[:, 38:39], rc2[:], op=mybir.AluOpType.mult
                )
                # sigmoid(L) = exp(-ln(1+exp(-L))) with only exp/ln;
                # L = t1 - t2n folds into the first exp's scale+bias.
                osb = spool.tile([NI, NCAPS], F32)
                nc.scalar.activation(
                    osb[:], t1[:], mybir.ActivationFunctionType.Exp,
                    scale=-1.0, bias=t2n[:],
                )
                nc.scalar.activation(
                    osb[:], osb[:], mybir.ActivationFunctionType.Ln,
                    bias=ones128[0:NI, :],
                )
                nc.scalar.activation(
                    osb[:], osb[:], mybir.ActivationFunctionType.Exp, scale=-1.0
                )
                nc.scalar.dma_start(OUT[:], osb[:])

            if loop_n == 1:
                body()
            else:
                with tc.For_i(0, loop_n, 1):
                    body()

    nc.compile()
    _CACHE[key] = nc
    return nc


def _fold_weights(Wp, bp, Wa, ba, Q, Wk, bk, Wv, bv, Wl, bl):
    f = lambda t: np.asarray(t, np.float64)
    Wp, bp, Wa, ba, Q, Wk, bk, Wv, bv, Wl, bl = map(
        f, (Wp, bp, Wa, ba, Q, Wk, bk, Wv, bv, Wl, bl)
    )
    wl = Wl[:, 0]
    WK = Wp.T @ Wk[:256]
    wvl_cap = Wv[:256] @ wl
    a, b = Wv[256] @ wl, Wv[257] @ wl

    W_all = np.zeros((CIN + 3, 66), np.float64)
    W_all[:CIN, :64] = WK
    W_all[:CIN, 64] = Wp.T @ wvl_cap
    W_all[:CIN, 65] = Wa[0]
    W_all[CIN + 0, :64] = Wk[256] / 64.0
    W_all[CIN + 1, :64] = Wk[257] / 64.0
    W_all[CIN + 2, :64] = bp @ Wk[:256] + bk
    W_all[CIN + 0, 64] = a / 64.0
    W_all[CIN + 1, 64] = b / 64.0
    W_all[CIN + 2, 64] = bp @ wvl_cap + bv @ wl
    W_all[CIN + 2, 65] = ba[0]

    c = np.arange(NCELL)
    y64 = (c // 64) / 64.0
    x64 = (c % 64) / 64.0
    wcorr = -(a * y64 + b * x64 - bl[0])
    WC2 = np.empty((128, 2 * NCK), np.float64)
    WC2[:, 0::2] = wcorr.reshape(NCK, 128).T
    WC2[:, 1::2] = 1.0

    bf = ml_dtypes.bfloat16
    # Fold Q into the channel weights: 21 useful output cols
    # [19 scores | vl | alog], padded to 32 for the fp8 DoubleRow LDW
    # restriction, scaled x64 into fp8 e4m3's normal range (descaled on chip).
    Wq = np.zeros((CIN + 3, 32))
    Wq[:, 0:NCAPS] = W_all[:, 0:64] @ (Q.T / 8.0)
    Wq[:, NCAPS : NCAPS + 2] = W_all[:, 64:66]
    WSW = np.ascontiguousarray(
        (Wq[:CIN] * 64.0)
        .reshape(5, 2, 128, 32)
        .transpose(2, 0, 1, 3)
        .reshape(128, 5 * 2 * 32)
    ).astype(ml_dtypes.float8_e4m3)
    c = np.arange(NCELL)
    C3v = np.stack([c // 64, c % 64, np.ones(NCELL)])
    C3M1 = (Wq[CIN:, 0:21].T @ C3v).astype(bf)
    return (
        WSW,
        C3M1,
        WC2.astype(bf),
    )


def _make_occt(point_lists):
    """[128 cells-in-chunk, chunk*32+instance] occupancy, per image."""
    bf = ml_dtypes.bfloat16
    pts = np.asarray(point_lists).astype(np.int64)  # [B, NI, 2, NPTS]
    ds = pts // 16
    keys = ds[:, :, 0] * 64 + ds[:, :, 1]  # [B, NI, NPTS]
    occ = np.zeros((B, NI, NCELL), np.float32)
    bi = np.arange(B)[:, None, None]
    ii = np.arange(NI)[None, :, None]
    occ[bi, ii, keys] = 1.0
    occt = np.ascontiguousarray(
        occ.reshape(B, NI, NCK, 128).transpose(0, 3, 2, 1).reshape(B, 128, NCK * NI)
    ).astype(bf)
    return occt


def _make_in_maps(
    feature_output, Wp, bp, Wa, ba, Q, Wk, bk, Wv, bv, Wl, bl, point_lists
):
    WSW, C3M1, WC2 = _fold_weights(Wp, bp, Wa, ba, Q, Wk, bk, Wv, bv, Wl, bl)

    # fp8 pair layout: [5 pairs, 128 part, 2 sub, 4096 cells] -> [640, 8192]
    fo = (
        np.asarray(feature_output, np.float32)
        .reshape(B, 5, 2, 128, NCELL)
        .transpose(0, 1, 3, 2, 4)
        .reshape(B, 5 * 128, 2 * NCELL)
        .astype(ml_dtypes.float8_e4m3)
    )
    occt = _make_occt(point_lists)

    return [
        {
            "X": fo[i],
            "WSW": WSW,
            "C3M1": C3M1,
            "WC2": WC2,
            "OCCT": occt[i],
        }
        for i in range(B)
    ]


def kernel(
    feature_output, Wp, bp, Wa, ba, Q, Wk, bk, Wv, bv, Wl, bl, point_lists
):
    nc = _build_nc()
    in_maps = _make_in_maps(
        feature_output, Wp, bp, Wa, ba, Q, Wk, bk, Wv, bv, Wl, bl, point_lists
    )
    res = run_bass_kernel_spmd(nc, in_maps, core_ids=list(range(B)))
    return np.stack([res.results[i]["OUT"] for i in range(B)]).astype(np.float32)
